# revision 20
# baseline (speedup 1.0000x reference)
"""Trainium2 Bass kernel for dynamic-depthwise + static conv module.

Computation (per batch b, channel c):
  hid  = leaky_relu(k_v @ W1.T, 0.1)
  kern = (hid @ W2.T).reshape(b*c, 3, 3)        # per-(b,c) dynamic 3x3
  dyn  = leaky_relu(depthwise3x3(x, kern), 0.1)
  res  = conv3x3(x, conv_w) + conv_b
  out  = dyn + res

Sharding: pure data-parallel, B=16 over 8 cores (2 batches/core).

Per-core "dy-pair" layout (bf16 conv path):
  For each batch b, one SBUF buffer xv_b [128 part, 194*194 bf16]:
    parts 0-63  = zero-padded x[b] (one channel per partition)
    parts 64-127 = the same, shifted up one padded row
  Both halves (padding and shift included) are prepared host-side in
  DRAM, so the load is a plain contiguous 128-partition bf16 DMA.
  A matmul stream at row offset d provides TWO dy taps at once
  (dy=d on parts 0-63, dy=d+1 on parts 64-127), so each 2-row output
  tile needs only 6 K=128 streams (d in {-1,+1} x dx in {-1,0,1})
  instead of 9 K=64 streams per batch.
  Batches interleave tile-by-tile with mirrored PSUM layouts
  (b0: static on parts 0-63, dyn on 64-127; b1 swapped), so the
  staging tiles fill all 128 partitions and the output store is one
  128-partition DMA.  ACT evicts dyn with Prelu (bf16), DVE adds bias
  to static (bf16), SWDGE accumulate-DMA folds dyn onto the opposite
  half, DMA to HBM (bf16; host upconverts).
"""
import numpy as np

import concourse.bass as bass
import concourse.tile as tile
import concourse.mybir as mybir

F32 = mybir.dt.float32
F32R = mybir.dt.float32r
BF16 = mybir.dt.bfloat16

B, C, H, W = 16, 64, 192, 192
NCORES = 8
BLOC = B // NCORES          # batches per core
WP = W + 2                  # padded row width
HP = H + 2
PADQ = WP * HP
G = 4                       # guard elems each side of padded buffer
HW = H * W
NTILE = 388                 # 2 padded rows per tile
NT = H // 2                 # 96 tiles per batch

# (row-offset d, dx): stream d carries taps dy=d (parts 0-63) and
# dy=d+1 (parts 64-127; zero weights when d=+1).
STREAMS = [(d, dx) for d in (-1, 1) for dx in (-1, 0, 1)]


def _legalize_waits(nc, max_waits=1, evsem_waits=2):
    """This walrus build rejects >1 sync wait on most instructions (2 on
    EventSemaphore). Spill excess waits onto same-engine EventSemaphores
    placed immediately before the instruction."""
    for f in nc.m.functions:
        for bb in f.blocks:
            new_insts = []
            for inst in bb.instructions:
                si = inst.sync_info
                if si is not None and si.on_wait and len(si.on_wait) > max_waits:
                    waits = list(si.on_wait)
                    keep = waits[-max_waits:]
                    spill = waits[:-max_waits]
                    while spill:
                        chunk, spill = spill[:evsem_waits], spill[evsem_waits:]
                        ev = mybir.InstEventSemaphore(
                            name=nc.get_next_instruction_name(),
                            engine=inst.engine,
                            ins=[],
                            outs=[],
                            sync_info=mybir.SyncInfo(on_wait=chunk, on_update=[]),
                        )
                        nc.register_instruction(ev)
                        new_insts.append(ev)
                    inst.sync_info = mybir.SyncInfo(
                        on_wait=keep, on_update=list(si.on_update or [])
                    )
                new_insts.append(inst)
            bb.instructions[:] = new_insts


def _build_nc(loopk=1, xrep=False, hwloop=False):
    nc = bass.Bass()
    xsd = [nc.dram_tensor(f"xs{b}", (128, PADQ), BF16, kind="ExternalInput")
           for b in range(BLOC)]
    mlpw = nc.dram_tensor("mlpw", (64, BLOC + 64 + 576), F32R,
                          kind="ExternalInput")
    wstat = nc.dram_tensor("wstat", (128, BLOC * 6 * 128), BF16,
                           kind="ExternalInput")
    ident = nc.dram_tensor("ident", (128, 64), BF16, kind="ExternalInput")
    biasd = nc.dram_tensor("biasd", (128, 1), F32, kind="ExternalInput")
    out = nc.dram_tensor("out", (128, HW), BF16, kind="ExternalOutput")

    with tile.TileContext(nc) as tc:
        with (
            tc.tile_pool(name="big", bufs=1) as big,
            tc.tile_pool(name="wpool", bufs=1) as wpool,
            tc.tile_pool(name="work", bufs=3) as work,
        ):
            # ---- persistent tiles ----
            xv = [big.tile([128, G + PADQ + G], BF16, tag=f"xv{b}",
                           name=f"xv{b}") for b in range(BLOC)]
            wb = [wpool.tile([128, 6 * 128], BF16, tag=f"wb{b}",
                             name=f"wb{b}") for b in range(BLOC)]
            id_t = wpool.tile([128, 64], BF16, tag="id_t")
            bias_t = wpool.tile([128, 1], F32, tag="bias_t")
            mlpw_t = wpool.tile([64, BLOC + 64 + 576], F32R, tag="mlpw_t")
            kvT_t = mlpw_t[:, 0:BLOC]
            w1t_t = mlpw_t[:, BLOC:BLOC + 64]
            w2t_t = mlpw_t[:, BLOC + 64:BLOC + 64 + 576]
            kern_flat = wpool.tile([BLOC, 576], F32, tag="kern_flat")
            k_rep = [wpool.tile([128, 9], F32, tag=f"k_rep{b}",
                                name=f"k_rep{b}") for b in range(BLOC)]
            hidT = wpool.tile([64, BLOC], F32R, tag="hidT")

            # pre-warm the ACT Prelu table while DMAs run (no input deps)
            warm = wpool.tile([1, 1], F32, tag="warm")
            nc.vector.memset(warm[:], 0.0)
            nc.scalar.activation(warm[:], warm[:],
                                 mybir.ActivationFunctionType.Prelu, alpha=0.1)

            # ---- small constant / weight loads (SP queue, idle here) ----
            nc.sync.dma_start(mlpw_t[:], mlpw[:])
            for b in range(BLOC):
                nc.sync.dma_start(wb[b][:], wstat[:, b * 768:(b + 1) * 768])
            nc.sync.dma_start(id_t[:], ident[:])
            nc.sync.dma_start(bias_t[:], biasd[:])

            # guard zeros (padding itself is baked into the DRAM layout)
            for b in range(BLOC):
                xbv = xv[b].bitcast(mybir.dt.uint16)
                nc.vector.memset(xbv[:, 0:G], 0)
                nc.vector.memset(xbv[:, G + PADQ:G + PADQ + G], 0)

            # ---- MLP: kern = (lrelu(k_v @ W1.T) @ W2.T), f32r ----
            with tc.tile_pool(name="pmlp", bufs=2, space="PSUM") as pmlp:
                p_hid = pmlp.tile([64, 512], F32, tag="pmlp")
                nc.tensor.matmul(p_hid[0:64, 0:BLOC], w1t_t, kvT_t,
                                 start=True, stop=True)
                nc.scalar.activation(hidT[:], p_hid[0:64, 0:BLOC],
                                     mybir.ActivationFunctionType.Prelu,
                                     alpha=0.1)
                p_k1 = pmlp.tile([64, 512], F32, tag="pmlp")
                p_k2 = pmlp.tile([64, 512], F32, tag="pmlp")
                nc.tensor.matmul(p_k1[0:BLOC, 0:288], hidT[:], w2t_t[:, 0:288],
                                 start=True, stop=True)
                nc.tensor.matmul(p_k2[0:BLOC, 0:288], hidT[:],
                                 w2t_t[:, 288:576], start=True, stop=True)
                nc.scalar.copy(kern_flat[:, 0:288], p_k1[0:BLOC, 0:288])
                nc.scalar.copy(kern_flat[:, 288:576], p_k2[0:BLOC, 0:288])

            # k_rep[b]: kern[b] replicated on both partition halves
            for b in range(BLOC):
                for h in range(2):
                    nc.sync.dma_start(k_rep[b][h * 64:(h + 1) * 64, :],
                                      kern_flat[b:b + 1, :])
            # diag fills. b0: dyn cols 64-127; b1 (mirrored): dyn cols 0-63.
            for b in range(BLOC):
                dyn0 = 64 if b == 0 else 0
                for si, (d, dx) in enumerate(STREAMS):
                    j = (d + 1) * 3 + (dx + 1)
                    nc.vector.tensor_scalar(
                        wb[b][0:64, si * 128 + dyn0:si * 128 + dyn0 + 64],
                        id_t[0:64, :], k_rep[b][0:64, j:j + 1], None,
                        op0=mybir.AluOpType.mult)
                    if d == -1:
                        j2 = (d + 2) * 3 + (dx + 1)
                        nc.vector.tensor_scalar(
                            wb[b][64:128, si * 128 + dyn0:si * 128 + dyn0 + 64],
                            id_t[64:128, :], k_rep[b][64:128, j2:j2 + 1],
                            None, op0=mybir.AluOpType.mult)

            # ---- x loads: contiguous 128-partition bf16 chunks ----
            NCHUNK = 6
            ROWS_PER_CHUNK = (HP + NCHUNK - 1) // NCHUNK   # 33 padded rows

            def load_x_chunk(c):
                if c >= BLOC * NCHUNK:
                    if not xrep:
                        return
                    c = c % (BLOC * NCHUNK)
                b, cc = divmod(c, NCHUNK)
                r0 = cc * ROWS_PER_CHUNK
                r1 = min(HP, r0 + ROWS_PER_CHUNK)
                nc.scalar.dma_start(xv[b][:, G + r0 * WP:G + r1 * WP],
                                    xsd[b][:, r0 * WP:r1 * WP])

            GT = 8
            NG = NT // GT
            PREFETCH = 2
            pmain_ctx = tc.tile_pool(name="pmain", bufs=8, space="PSUM")
            pmain = pmain_ctx.__enter__()

            def emit_group(g, do_load, rep=0):
                # each group consumes half a chunk of rows; stay ahead
                if do_load and g % 2 == 0:
                    for b in range(BLOC):
                        load_x_chunk(rep * BLOC * NCHUNK
                                     + b * NCHUNK + g // 2 + PREFETCH)
                dyn_s = work.tile([128, GT, NTILE], BF16, tag="dyn_s", bufs=2)
                st = work.tile([128, GT, 2, W], BF16, tag="st", bufs=2)
                for gi in range(GT):
                    ti = g * GT + gi
                    hp0 = 1 + 2 * ti
                    qs = hp0 * WP
                    for b in range(BLOC):
                        # b0: static -> parts 0-63, dyn -> 64-127; b1 mirrored
                        s0, d0 = (0, 64) if b == 0 else (64, 0)
                        pbf = pmain.tile([128, 512], F32, tag="pb")
                        pb = pbf[:, 0:NTILE]
                        for si, (d, dx) in enumerate(STREAMS):
                            base = G + qs + d * WP + dx
                            nc.tensor.matmul(
                                pb[:], wb[b][:, si * 128:(si + 1) * 128],
                                xv[b][:, base:base + NTILE],
                                start=(si == 0), stop=(si == 5))
                        # dyn half: Prelu -> padded bf16 staging
                        nc.scalar.activation(
                            dyn_s[d0:d0 + 64, gi, :], pb[d0:d0 + 64, :],
                            mybir.ActivationFunctionType.Prelu, alpha=0.1)
                        # fold leaky(dyn) onto the static half on the PE
                        nc.tensor.matmul(
                            pb[s0:s0 + 64, :], id_t[d0:d0 + 64, :],
                            dyn_s[d0:d0 + 64, gi, :],
                            start=False, stop=True, skip_group_check=True,
                            tile_position=(d0, s0))
                        # static+dyn half: add bias -> compact bf16 staging
                        pvs = pb[s0:s0 + 64, :].rearrange(
                            "p (r w) -> p r w", w=WP)[:, :, 1:1 + W]
                        nc.vector.tensor_scalar(st[s0:s0 + 64, gi, :, :], pvs,
                                                bias_t[s0:s0 + 64, :], None,
                                                op0=mybir.AluOpType.add)
                # store the group to HBM
                h0 = 2 * GT * g
                if g < NG - 1:
                    nc.sync.dma_start(out[:, h0 * W:(h0 + 2 * GT) * W], st[:])
                else:
                    # finer-grained tail: per-tile-pair out DMAs
                    for gi in range(0, GT, 2):
                        hh = h0 + 2 * gi
                        nc.sync.dma_start(out[:, hh * W:(hh + 4) * W],
                                          st[:, gi:gi + 2, :, :])

            for c in range(PREFETCH):
                load_x_chunk(0 * NCHUNK + c)
                load_x_chunk(1 * NCHUNK + c)
            for rep in range(loopk):
                for g in range(NG):
                    emit_group(g, do_load=True, rep=rep)
            pmain_ctx.__exit__(None, None, None)

    _legalize_waits(nc)
    return nc


_NC_CACHE = {}


def _get_nc(loopk=1, xrep=False, hwloop=False):
    key = (loopk, xrep, hwloop)
    if key not in _NC_CACHE:
        _NC_CACHE[key] = _build_nc(loopk, xrep, hwloop)
    return _NC_CACHE[key]


def make_in_maps(x, k_v, W1, W2, conv_w, conv_b):
    import ml_dtypes

    # host-side weight layout prep (parameters only)
    def stat_blocks(dyn0):
        blocks = []
        for d, dx in STREAMS:
            lo = conv_w[:, :, d + 1, dx + 1].T          # [ci, co] tap dy=d
            if d == -1:
                hi = conv_w[:, :, d + 2, dx + 1].T      # tap dy=d+1
            else:
                hi = np.zeros((64, 64), np.float32)
            stat = np.concatenate([lo, hi], axis=0)     # [128, 64]
            zer = np.zeros((128, 64), np.float32)
            blk = (np.concatenate([stat, zer], axis=1) if dyn0 == 64
                   else np.concatenate([zer, stat], axis=1))
            blocks.append(blk)
        return np.concatenate(blocks, axis=1)           # [128, 768]

    wstat = np.concatenate([stat_blocks(64), stat_blocks(0)],
                           axis=1).astype(ml_dtypes.bfloat16)
    ident = np.tile(np.eye(64, dtype=np.float32), (2, 1)).astype(
        ml_dtypes.bfloat16)
    biasd = np.tile(conv_b, 2)[:, None].astype(np.float32)
    w1t = W1.T.copy()                               # [64, 64]
    w2t = W2.T.copy()                               # [64, 576]

    # x: padded + one-row-shifted dup halves, bf16, baked host-side
    xb = x.astype(ml_dtypes.bfloat16)               # (B, C, H, W)
    in_maps = []
    for cidx in range(NCORES):
        im = {}
        for b in range(BLOC):
            xi = xb[cidx * BLOC + b]                # (C, H, W)
            xpad = np.zeros((C, HP, WP), ml_dtypes.bfloat16)
            xpad[:, 1:1 + H, 1:1 + W] = xi
            dup = np.zeros((C, HP, WP), ml_dtypes.bfloat16)
            dup[:, 0:H, 1:1 + W] = xi
            im[f"xs{b}"] = np.concatenate(
                [xpad.reshape(C, PADQ), dup.reshape(C, PADQ)], axis=0)
        kvT = k_v[cidx * BLOC:(cidx + 1) * BLOC].T.copy()
        im["mlpw"] = np.concatenate([kvT, w1t, w2t], axis=1)
        im["wstat"] = wstat
        im["ident"] = ident
        im["biasd"] = biasd
        in_maps.append(im)
    return in_maps


def expected_core0(full):
    return full[0:BLOC].reshape(128, HW)


def kernel(x, k_v, W1, W2, conv_w, conv_b):
    from concourse.bass_utils import run_bass_kernel_spmd

    x = np.ascontiguousarray(x, dtype=np.float32)
    k_v = np.ascontiguousarray(k_v, dtype=np.float32)
    W1 = np.ascontiguousarray(W1, dtype=np.float32)
    W2 = np.ascontiguousarray(W2, dtype=np.float32)
    conv_w = np.ascontiguousarray(conv_w, dtype=np.float32)
    conv_b = np.ascontiguousarray(conv_b, dtype=np.float32)

    in_maps = make_in_maps(x, k_v, W1, W2, conv_w, conv_b)
    nc = _get_nc()
    res = run_bass_kernel_spmd(nc, in_maps, core_ids=list(range(NCORES)))
    out = np.empty((B, C, H, W), dtype=np.float32)
    for c in range(NCORES):
        out[c * BLOC:(c + 1) * BLOC] = res.results[c]["out"].astype(
            np.float32).reshape(BLOC, C, H, W)
    return out
